# revision 36
# baseline (speedup 1.0000x reference)
import numpy as np

# DiffSortNet bitonic differentiable sort, B=8, N=1024, 55 layers.
# Strategy: data-parallel (one batch element per core). Per core, the
# permutation matrix is kept TRANSPOSED in SBUF as 8 tiles XT_f [128,1024]
# (partition = column index of X). Each layer is a per-column blend
#   new_col_c = al_c * col_c + (1-al_c) * col_{c^m}
# with pair-symmetric alpha, which in the transposed layout is a [128,128]
# matmul per tile with the symmetric blend matrix Wt = al*D + P  (D = I-P,
# P the XOR-m permutation), or for m>=128 a pair of diagonal-matrix matmuls
# across tile pairs. Alphas come from a tiny [8,128] x-chain computed on
# DVE/ACT and transposed to per-partition scale vectors via the PE.

N = 1024
NL = 55
NCORES = 8

_CACHE = {}
LAST_EXEC_NS = [None]
LAST_RESULTS = [None]
LAST_INMAP = [None]


def _derive_layers(ia, ib):
    ms = (ia ^ ib)
    layers = []
    prev_m = None
    cur_block_top = None
    for t in range(ia.shape[0]):
        mv = np.unique(ms[t])
        assert mv.size == 1, "non-uniform stride in layer"
        m = int(mv[0])
        if prev_m is None or m > prev_m:
            cur_block_top = m
        s_out = 2 * cur_block_top
        w = min(max(s_out, 128), N)
        layers.append((m, w))
        prev_m = m
    return layers


def _build_masks(ia, ib, layers):
    # sgn10[t, c] = +10 if ib of c's pair sits at (c|m) else -10
    # pm[t, c]    = +1 if c is the low member of its pair else -1
    sgn = np.empty((NL, N), np.float32)
    pm = np.empty((NL, N), np.float32)
    for t, (m, _) in enumerate(layers):
        a, b = ia[t], ib[t]
        hi = np.maximum(a, b)
        val = np.where(b == hi, 10.0, -10.0).astype(np.float32)
        s = np.empty(N, np.float32)
        s[a] = val
        s[b] = val
        sgn[t] = s
        c = np.arange(N)
        pm[t] = np.where((c & m) == 0, 1.0, -1.0)
    # reshape to [8, NL*128] with c = f*128 + p
    sgn_r = sgn.reshape(NL, 8, 128).transpose(1, 0, 2).reshape(8, NL * 128).copy()
    pm_r = pm.reshape(NL, 8, 128).transpose(1, 0, 2).reshape(8, NL * 128).copy()
    return sgn_r, pm_r


def _build_program(layers):
    import os
    import concourse.bacc as bacc
    import concourse.mybir as mybir
    from concourse.tile import TileContext

    mode = os.environ.get("DSORT_MODE", "full")

    F32 = mybir.dt.float32
    F32R = mybir.dt.float32r
    AF = mybir.ActivationFunctionType
    ALU = mybir.AluOpType

    small_ms = sorted({m for (m, _) in layers if m < 128})
    tmap = {m: i for i, m in enumerate(small_ms)}
    NT = len(small_ms)

    nc = bacc.Bacc()
    vec_d = nc.dram_tensor("vec", [8, 128], F32, kind="ExternalInput")
    sgn_d = nc.dram_tensor("sgn", [8, NL * 128], F32, kind="ExternalInput")
    pm_d = nc.dram_tensor("pm", [8, NL * 128], F32, kind="ExternalInput")
    p_d = nc.dram_tensor("pmask", [128, NT * 128], F32, kind="ExternalInput")
    d_d = nc.dram_tensor("dmask", [128, NT * 128], F32, kind="ExternalInput")
    id128_d = nc.dram_tensor("id128", [128, 128], F32, kind="ExternalInput")
    id8_d = nc.dram_tensor("id8", [8, 8], F32, kind="ExternalInput")
    mk_d = nc.dram_tensor("mk", [8, 3 * 8], F32, kind="ExternalInput")
    xs_d = nc.dram_tensor("xs", [8, 128], F32, kind="ExternalOutput")
    xt_d = nc.dram_tensor("xt", [N, N], F32, kind="ExternalOutput")

    with TileContext(nc) as tc:
        with tc.tile_pool(name="cst", bufs=1) as cst, \
             tc.tile_pool(name="xp", bufs=1) as xp, \
             tc.tile_pool(name="ch", bufs=4) as ch, \
             tc.tile_pool(name="wp", bufs=8) as wp, \
             tc.tile_pool(name="psmm", bufs=4, space="PSUM") as psmm, \
             tc.tile_pool(name="pstr", bufs=2, space="PSUM") as pstr:

            sgn_sb = cst.tile([8, NL * 128], F32, tag="sgn")
            pm_sb = cst.tile([8, NL * 128], F32, tag="pmm")
            p_sb = cst.tile([128, NT * 128], F32R, tag="pall")
            dm_sb = cst.tile([128, NT * 128], F32R, tag="dall")
            idr_sb = cst.tile([128, 128], F32R, tag="idr")
            id8_sb = cst.tile([8, 8], F32, tag="id8")
            mk_sb = cst.tile([8, 3 * 8], F32, tag="mk")
            nc.sync.dma_start(out=mk_sb[:], in_=mk_d[:])
            epsb = cst.tile([8, 1], F32, tag="epsb")
            nc.gpsimd.memset(epsb[:], 1e-38)
            nc.sync.dma_start(out=sgn_sb[:], in_=sgn_d[:])
            nc.sync.dma_start(out=pm_sb[:], in_=pm_d[:])
            nc.sync.dma_start(out=p_sb[:], in_=p_d[:].bitcast(F32R))
            nc.sync.dma_start(out=dm_sb[:], in_=d_d[:].bitcast(F32R))
            nc.sync.dma_start(out=idr_sb[:], in_=id128_d[:].bitcast(F32R))
            nc.sync.dma_start(out=id8_sb[:], in_=id8_d[:])

            xt = [xp.tile([128, N], F32R, tag=f"X{f}", name=f"X{f}") for f in range(8)]
            zsc = cst.tile([128, N], F32, tag="zsc")
            nc.gpsimd.memset(zsc[:], 0.0)
            for f in range(8):
                if f % 2 == 0:
                    nc.gpsimd.tensor_copy(xt[f][:], zsc[:])
                elif f % 4 == 1:
                    nc.vector.tensor_copy(xt[f][:], zsc[:])
                else:
                    nc.scalar.activation(xt[f][:], zsc[:], AF.Copy)
                nc.sync.dma_start(
                    out=xt[f][:, f * 128:(f + 1) * 128],
                    in_=id128_d[:].bitcast(F32R),
                )

            x8 = ch.tile([8, 128], F32, tag="x8", bufs=4)
            nc.gpsimd.dma_start(out=x8[:], in_=vec_d[:])

            alTs = {}

            alTc = cst.tile([128, 8], F32, tag="alTc")
            nc.gpsimd.memset(alTc[:], 0.5)

            def chain(t):
                nonlocal x8
                m, w = layers[t]
                if mode == "xwork":
                    alTs[t] = alTc
                    if m >= 128:
                        alTs[(t, '1m')] = alTc
                    return
                dfull = ch.tile([8, 128], F32, tag="dfull")
                if m < 128:
                    xv = x8[:].rearrange("f (g two m) -> f g two m", two=2, m=m)
                    dv = dfull[:].rearrange("f (g two m) -> f g two m", two=2, m=m)
                    hi = xv[:, :, 1:2, :].broadcast_to((8, 128 // (2 * m), 2, m))
                    lo = xv[:, :, 0:1, :].broadcast_to((8, 128 // (2 * m), 2, m))
                    nc.vector.tensor_tensor(dv[:, :, :, :], hi, lo, ALU.subtract)
                else:
                    k = m // 128
                    ki = {1: 0, 2: 1, 4: 2}[k]
                    dps = pstr.tile([8, 128], F32, tag="dps", bufs=1)
                    nc.tensor.matmul(dps[:], mk_sb[:, ki * 8:(ki + 1) * 8], x8[:], start=True, stop=True)
                    nc.vector.tensor_copy(dfull[:], dps[:])
                # critical recurrence: dfull -> Abs -> Ln -> Exp(.75) -> z3 -> sig- -> q -> x8n
                a1 = ch.tile([8, 128], F32, tag="a1")
                nc.scalar.activation(a1[:], dfull[:], AF.Abs)
                lg = ch.tile([8, 128], F32, tag="lg")
                nc.scalar.activation(lg[:], a1[:], AF.Ln, bias=epsb[:])
                w34 = ch.tile([8, 128], F32, tag="w34")
                nc.scalar.activation(w34[:], lg[:], AF.Exp, scale=0.75)
                # parallel branches (off critical path)
                s0m0 = ch.tile([8, 128], F32, tag="s0m0")
                nc.vector.tensor_tensor(s0m0[:], dfull[:], sgn_sb[:, t * 128:(t + 1) * 128], ALU.mult)
                s0m = ch.tile([8, 128], F32, tag="s0m")
                nc.scalar.activation(s0m[:], s0m0[:], AF.Sign)
                t1 = ch.tile([8, 128], F32, tag="t1")
                nc.vector.tensor_tensor(t1[:], dfull[:], pm_sb[:, t * 128:(t + 1) * 128], ALU.mult)
                # join
                z3 = ch.tile([8, 128], F32, tag="z3")
                nc.vector.tensor_tensor(z3[:], w34[:], s0m[:], ALU.mult)
                sigm = ch.tile([8, 128], F32, tag="sigm")
                nc.scalar.activation(sigm[:], z3[:], AF.Sigmoid, scale=-10.0)
                q = ch.tile([8, 128], F32, tag="q")
                nc.vector.tensor_tensor(q[:], t1[:], sigm[:], ALU.mult)
                x8n = ch.tile([8, 128], F32, tag="x8", bufs=4)
                nc.vector.tensor_tensor(x8n[:], x8[:], q[:], ALU.add)
                x8 = x8n
                # alpha for the X update (off recurrence)
                al8 = ch.tile([8, 128], F32, tag="al8")
                nc.scalar.activation(al8[:], z3[:], AF.Sigmoid, scale=10.0)
                pst = pstr.tile([128, 8], F32, tag="tr")
                nc.tensor.transpose(pst[:], al8[:], id8_sb[:])
                alT = ch.tile([128, 8], F32, tag="alT", bufs=6)
                nc.vector.tensor_copy(alT[:], pst[:])
                alTs[t] = alT
                if m >= 128:
                    alT1m = ch.tile([128, 8], F32, tag="alT1m")
                    nc.scalar.activation(alT1m[:], alT[:], AF.Copy, bias=1.0, scale=-1.0)
                    alTs[(t, '1m')] = alT1m

            cb_toggle = [0]
            fstate = {}

            def copyback(dst_ap, src_ap):
                if cb_toggle[0] % 4 != 3:
                    nc.vector.tensor_copy(dst_ap, src_ap)
                else:
                    nc.scalar.activation(dst_ap, src_ap, AF.Copy)
                cb_toggle[0] += 1

            def xwork(t):
                if mode == "chain":
                    alTs.pop(t, None)
                    alTs.pop((t, '1m'), None)
                    return
                m, w = layers[t]
                alT = alTs.pop(t)
                if m < 128 and w >= 512:
                    # fused: accumulate per-tile [128,128] products, apply once
                    first = (t == 0) or not (layers[t - 1][0] < 128 and layers[t - 1][1] >= 512)
                    last = (t == NL - 1) or not (layers[t + 1][0] < 128 and layers[t + 1][1] >= 512)
                    ti = tmap[m]
                    psl = p_sb[:, ti * 128:(ti + 1) * 128]
                    dsl = dm_sb[:, ti * 128:(ti + 1) * 128]
                    for f in range(8):
                        if first:
                            ff = wp.tile([128, 128], F32R, tag=f"F{f}", name=f"F{f}_{t}", bufs=2)
                            fstate[f] = ff
                            nc.vector.tensor_scalar(ff[:], dsl, alT[:, f:f + 1], None, ALU.mult)
                            nc.gpsimd.tensor_tensor(ff[:], ff[:], psl, ALU.add)
                        else:
                            wt = wp.tile([128, 128], F32R, tag="wt")
                            nc.vector.tensor_scalar(wt[:], dsl, alT[:, f:f + 1], None, ALU.mult)
                            nc.gpsimd.tensor_tensor(wt[:], wt[:], psl, ALU.add)
                            ps = psmm.tile([128, 128], F32, tag="mm")
                            nc.tensor.matmul(ps[:], wt[:], fstate[f][:], start=True, stop=True)
                            copyback(fstate[f][:], ps[:])
                        if last:
                            pst = psmm.tile([128, 128], F32R, tag="mm")
                            nc.tensor.transpose(pst[:], fstate[f][:], idr_sb[:])
                            ft = wp.tile([128, 128], F32R, tag="wt")
                            nc.vector.tensor_copy(ft[:], pst[:])
                            off = w * ((128 * f) // w)
                            for c0 in range(off, off + w, 512):
                                cw = min(512, off + w - c0)
                                ps = psmm.tile([128, cw], F32, tag="mm")
                                nc.tensor.matmul(ps[:], ft[:], xt[f][:, c0:c0 + cw], start=True, stop=True)
                                copyback(xt[f][:, c0:c0 + cw], ps[:])
                elif m < 128:
                    ti = tmap[m]
                    psl = p_sb[:, ti * 128:(ti + 1) * 128]
                    dsl = dm_sb[:, ti * 128:(ti + 1) * 128]
                    for f in range(8):
                        wt = wp.tile([128, 128], F32R, tag="wt")
                        nc.vector.tensor_scalar(wt[:], dsl, alT[:, f:f + 1], None, ALU.mult)
                        nc.gpsimd.tensor_tensor(wt[:], wt[:], psl, ALU.add)
                        off = w * ((128 * f) // w)
                        for c0 in range(off, off + w, 512):
                            cw = min(512, off + w - c0)
                            ps = psmm.tile([128, cw], F32, tag="mm")
                            nc.tensor.matmul(ps[:], wt[:], xt[f][:, c0:c0 + cw], start=True, stop=True)
                            copyback(xt[f][:, c0:c0 + cw], ps[:])
                else:
                    k = m // 128
                    alT1m = alTs.pop((t, '1m'))
                    done = set()
                    for f in range(8):
                        f2 = f ^ k
                        if f in done:
                            continue
                        done.update((f, f2))
                        v1a = wp.tile([128, 128], F32R, tag="wt")
                        v2a = wp.tile([128, 128], F32R, tag="wt")
                        v1b = wp.tile([128, 128], F32R, tag="wt")
                        v2b = wp.tile([128, 128], F32R, tag="wt")
                        nc.scalar.activation(v1a[:], idr_sb[:], AF.Copy, scale=alT[:, f:f + 1])
                        nc.vector.tensor_scalar(v2a[:], idr_sb[:], alT1m[:, f:f + 1], None, ALU.mult)
                        nc.scalar.activation(v1b[:], idr_sb[:], AF.Copy, scale=alT[:, f2:f2 + 1])
                        nc.vector.tensor_scalar(v2b[:], idr_sb[:], alT1m[:, f2:f2 + 1], None, ALU.mult)
                        off = w * ((128 * f) // w)
                        for c0 in range(off, off + w, 512):
                            cw = min(512, off + w - c0)
                            psa = psmm.tile([128, cw], F32, tag="mm")
                            nc.tensor.matmul(psa[:], v1a[:], xt[f][:, c0:c0 + cw], start=True, stop=False)
                            nc.tensor.matmul(psa[:], v2a[:], xt[f2][:, c0:c0 + cw], start=False, stop=True)
                            psb = psmm.tile([128, cw], F32, tag="mm")
                            nc.tensor.matmul(psb[:], v1b[:], xt[f2][:, c0:c0 + cw], start=True, stop=False)
                            nc.tensor.matmul(psb[:], v2b[:], xt[f][:, c0:c0 + cw], start=False, stop=True)
                            copyback(xt[f][:, c0:c0 + cw], psa[:])
                            copyback(xt[f2][:, c0:c0 + cw], psb[:])

            chain(0)
            chain(1)
            chain(2)
            for t in range(NL):
                if t + 3 < NL:
                    chain(t + 3)
                xwork(t)

            nc.sync.dma_start(out=xs_d[:], in_=x8[:])
            for f in range(8):
                nc.sync.dma_start(
                    out=xt_d[f * 128:(f + 1) * 128, :],
                    in_=xt[f][:].bitcast(F32),
                )

    nc.compile()
    return nc


def kernel(vectors, idx_a, idx_b):
    from concourse.bass_utils import run_bass_kernel_spmd

    vectors = np.asarray(vectors, dtype=np.float32)
    ia = np.asarray(idx_a).astype(np.int64)
    ib = np.asarray(idx_b).astype(np.int64)

    layers = _derive_layers(ia, ib)
    key = (ia.tobytes(), ib.tobytes())
    if key not in _CACHE:
        _CACHE[key] = _build_program(layers)
    nc = _CACHE[key]

    sgn_r, pm_r = _build_masks(ia, ib, layers)
    small_ms = sorted({m for (m, _) in layers if m < 128})
    NT = len(small_ms)
    pmask = np.zeros((128, NT, 128), np.float32)
    dmask = np.zeros((128, NT, 128), np.float32)
    eye = np.eye(128, dtype=np.float32)
    for i, m in enumerate(small_ms):
        p = np.arange(128)
        pmat = np.zeros((128, 128), np.float32)
        pmat[p, p ^ m] = 1.0
        pmask[:, i, :] = pmat
        dmask[:, i, :] = eye - pmat
    pmask = pmask.reshape(128, NT * 128)
    dmask = dmask.reshape(128, NT * 128)

    mk = np.zeros((3, 8, 8), np.float32)
    for ki, k in enumerate((1, 2, 4)):
        for f in range(8):
            mk[ki, f, f | k] += 1.0
            mk[ki, f, f & ~k] -= 1.0
    mk_t = mk.transpose(0, 2, 1).transpose(1, 0, 2).reshape(8, 3 * 8).copy()

    base = {
        "sgn": sgn_r,
        "pm": pm_r,
        "pmask": pmask,
        "dmask": dmask,
        "id128": eye,
        "id8": np.eye(8, dtype=np.float32),
        "mk": mk_t,
    }
    in_maps = []
    for c in range(NCORES):
        m = dict(base)
        m["vec"] = vectors[c].reshape(8, 128).copy()
        in_maps.append(m)

    LAST_INMAP[0] = in_maps[0]
    res = run_bass_kernel_spmd(nc, in_maps, list(range(NCORES)))
    LAST_EXEC_NS[0] = res.exec_time_ns
    LAST_RESULTS[0] = res

    x_out = np.empty((NCORES, N), np.float32)
    X_out = np.empty((NCORES, N, N), np.float32)
    for c in range(NCORES):
        x_out[c] = res.results[c]["xs"].reshape(N)
        X_out[c] = res.results[c]["xt"].T
    return x_out, X_out


# revision 39
# speedup vs baseline: 1.1461x; 1.1461x over previous
import numpy as np

# DiffSortNet bitonic differentiable sort, B=8, N=1024, 55 layers.
# Strategy: data-parallel (one batch element per core). Per core, the
# permutation matrix is kept TRANSPOSED in SBUF as 8 tiles XT_f [128,1024]
# (partition = column index of X). Each layer is a per-column blend
#   new_col_c = al_c * col_c + (1-al_c) * col_{c^m}
# with pair-symmetric alpha, which in the transposed layout is a [128,128]
# matmul per tile with the symmetric blend matrix Wt = al*D + P  (D = I-P,
# P the XOR-m permutation), or for m>=128 a pair of diagonal-matrix matmuls
# across tile pairs. Alphas come from a tiny [8,128] x-chain computed on
# DVE/ACT and transposed to per-partition scale vectors via the PE.

N = 1024
NL = 55
NCORES = 8

_CACHE = {}
LAST_EXEC_NS = [None]
LAST_RESULTS = [None]
LAST_INMAP = [None]


def _derive_layers(ia, ib):
    ms = (ia ^ ib)
    layers = []
    prev_m = None
    cur_block_top = None
    for t in range(ia.shape[0]):
        mv = np.unique(ms[t])
        assert mv.size == 1, "non-uniform stride in layer"
        m = int(mv[0])
        if prev_m is None or m > prev_m:
            cur_block_top = m
        s_out = 2 * cur_block_top
        w = min(max(s_out, 128), N)
        layers.append((m, w))
        prev_m = m
    return layers


def _build_masks(ia, ib, layers):
    # sgn10[t, c] = +10 if ib of c's pair sits at (c|m) else -10
    # pm[t, c]    = +1 if c is the low member of its pair else -1
    sgn = np.empty((NL, N), np.float32)
    pm = np.empty((NL, N), np.float32)
    for t, (m, _) in enumerate(layers):
        a, b = ia[t], ib[t]
        hi = np.maximum(a, b)
        val = np.where(b == hi, 10.0, -10.0).astype(np.float32)
        s = np.empty(N, np.float32)
        s[a] = val
        s[b] = val
        sgn[t] = s
        c = np.arange(N)
        pm[t] = np.where((c & m) == 0, 1.0, -1.0)
    # reshape to [128, NL*8]: row p, col (t, f), c = f*128 + p
    sgn_r = sgn.reshape(NL, 8, 128).transpose(2, 0, 1).reshape(128, NL * 8).copy()
    pm_r = pm.reshape(NL, 8, 128).transpose(2, 0, 1).reshape(128, NL * 8).copy()
    return sgn_r, pm_r


def _build_program(layers):
    import os
    import concourse.bacc as bacc
    import concourse.mybir as mybir
    from concourse.tile import TileContext

    mode = os.environ.get("DSORT_MODE", "full")

    F32 = mybir.dt.float32
    F32R = mybir.dt.float32r
    AF = mybir.ActivationFunctionType
    ALU = mybir.AluOpType

    small_ms = sorted({m for (m, _) in layers if m < 128})
    tmap = {m: i for i, m in enumerate(small_ms)}
    NT = len(small_ms)

    nc = bacc.Bacc()
    vec_d = nc.dram_tensor("vec", [128, 8], F32, kind="ExternalInput")
    sgn_d = nc.dram_tensor("sgn", [128, NL * 8], F32, kind="ExternalInput")
    pm_d = nc.dram_tensor("pm", [128, NL * 8], F32, kind="ExternalInput")
    p_d = nc.dram_tensor("pmask", [128, NT * 128], F32, kind="ExternalInput")
    d_d = nc.dram_tensor("dmask", [128, NT * 128], F32, kind="ExternalInput")
    id128_d = nc.dram_tensor("id128", [128, 128], F32, kind="ExternalInput")
    md_d = nc.dram_tensor("mdm", [128, NT * 128], F32, kind="ExternalInput")
    xs_d = nc.dram_tensor("xs", [128, 8], F32, kind="ExternalOutput")
    xt_d = nc.dram_tensor("xt", [N, N], F32, kind="ExternalOutput")

    with TileContext(nc) as tc:
        with tc.tile_pool(name="cst", bufs=1) as cst, \
             tc.tile_pool(name="xp", bufs=1) as xp, \
             tc.tile_pool(name="ch", bufs=4) as ch, \
             tc.tile_pool(name="wp", bufs=8) as wp, \
             tc.tile_pool(name="psmm", bufs=4, space="PSUM") as psmm, \
             tc.tile_pool(name="pstr", bufs=2, space="PSUM") as pstr:

            sgn_sb = cst.tile([128, NL * 8], F32, tag="sgn")
            pm_sb = cst.tile([128, NL * 8], F32, tag="pmm")
            p_sb = cst.tile([128, NT * 128], F32R, tag="pall")
            dm_sb = cst.tile([128, NT * 128], F32R, tag="dall")
            idr_sb = cst.tile([128, 128], F32R, tag="idr")
            md_sb = cst.tile([128, NT * 128], F32, tag="mdm")
            nc.sync.dma_start(out=md_sb[:], in_=md_d[:])
            epsb = cst.tile([128, 1], F32, tag="epsb")
            nc.gpsimd.memset(epsb[:], 1e-38)
            nc.sync.dma_start(out=sgn_sb[:], in_=sgn_d[:])
            nc.sync.dma_start(out=pm_sb[:], in_=pm_d[:])
            nc.sync.dma_start(out=p_sb[:], in_=p_d[:].bitcast(F32R))
            nc.sync.dma_start(out=dm_sb[:], in_=d_d[:].bitcast(F32R))
            nc.sync.dma_start(out=idr_sb[:], in_=id128_d[:].bitcast(F32R))

            xt = [xp.tile([128, N], F32R, tag=f"X{f}", name=f"X{f}") for f in range(8)]
            zsc = cst.tile([128, N], F32, tag="zsc")
            nc.gpsimd.memset(zsc[:], 0.0)
            for f in range(8):
                if f % 2 == 0:
                    nc.gpsimd.tensor_copy(xt[f][:], zsc[:])
                elif f % 4 == 1:
                    nc.vector.tensor_copy(xt[f][:], zsc[:])
                else:
                    nc.scalar.activation(xt[f][:], zsc[:], AF.Copy)
                nc.sync.dma_start(
                    out=xt[f][:, f * 128:(f + 1) * 128],
                    in_=id128_d[:].bitcast(F32R),
                )

            x8 = ch.tile([128, 8], F32, tag="x8", bufs=4)
            nc.gpsimd.dma_start(out=x8[:], in_=vec_d[:])

            alTs = {}

            alTc = cst.tile([128, 8], F32, tag="alTc")
            nc.gpsimd.memset(alTc[:], 0.5)

            def chain(t):
                nonlocal x8
                m, w = layers[t]
                if mode == "xwork":
                    alTs[t] = alTc
                    if m >= 128:
                        alTs[(t, '1m')] = alTc
                    return
                dfull = ch.tile([128, 8], F32, tag="dfull")
                if m < 128:
                    # pair difference across partitions via const matrix matmul
                    ti = tmap[m]
                    dps = pstr.tile([128, 8], F32, tag="dps", bufs=1)
                    nc.tensor.matmul(dps[:], md_sb[:, ti * 128:(ti + 1) * 128], x8[:], start=True, stop=True)
                    nc.vector.tensor_copy(dfull[:], dps[:])
                else:
                    k = m // 128
                    xv = x8[:].rearrange("p (g two k) -> p g two k", two=2, k=k)
                    dv = dfull[:].rearrange("p (g two k) -> p g two k", two=2, k=k)
                    hi = xv[:, :, 1:2, :].broadcast_to((128, 8 // (2 * k), 2, k))
                    lo = xv[:, :, 0:1, :].broadcast_to((128, 8 // (2 * k), 2, k))
                    nc.vector.tensor_tensor(dv[:, :, :, :], hi, lo, ALU.subtract)
                # critical recurrence: dfull -> Abs -> Ln -> Exp(.75) -> z3 -> sig- -> q -> x8n
                a1 = ch.tile([128, 8], F32, tag="a1")
                nc.scalar.activation(a1[:], dfull[:], AF.Abs)
                lg = ch.tile([128, 8], F32, tag="lg")
                nc.scalar.activation(lg[:], a1[:], AF.Ln, bias=epsb[:])
                w34 = ch.tile([128, 8], F32, tag="w34")
                nc.scalar.activation(w34[:], lg[:], AF.Exp, scale=0.75)
                # parallel branches (off critical path)
                s0m0 = ch.tile([128, 8], F32, tag="s0m0")
                nc.vector.tensor_tensor(s0m0[:], dfull[:], sgn_sb[:, t * 8:(t + 1) * 8], ALU.mult)
                s0m = ch.tile([128, 8], F32, tag="s0m")
                nc.scalar.activation(s0m[:], s0m0[:], AF.Sign)
                t1 = ch.tile([128, 8], F32, tag="t1")
                nc.vector.tensor_tensor(t1[:], dfull[:], pm_sb[:, t * 8:(t + 1) * 8], ALU.mult)
                # join
                z3 = ch.tile([128, 8], F32, tag="z3")
                nc.vector.tensor_tensor(z3[:], w34[:], s0m[:], ALU.mult)
                sigm = ch.tile([128, 8], F32, tag="sigm")
                nc.scalar.activation(sigm[:], z3[:], AF.Sigmoid, scale=-10.0)
                q = ch.tile([128, 8], F32, tag="q")
                nc.vector.tensor_tensor(q[:], t1[:], sigm[:], ALU.mult)
                x8n = ch.tile([128, 8], F32, tag="x8", bufs=4)
                nc.vector.tensor_tensor(x8n[:], x8[:], q[:], ALU.add)
                x8 = x8n
                # alpha for the X update (off recurrence) -- already [128, 8]
                alT = ch.tile([128, 8], F32, tag="alT", bufs=6)
                nc.scalar.activation(alT[:], z3[:], AF.Sigmoid, scale=10.0)
                alTs[t] = alT
                if m >= 128:
                    alT1m = ch.tile([128, 8], F32, tag="alT1m")
                    nc.scalar.activation(alT1m[:], sigm[:], AF.Copy)
                    alTs[(t, '1m')] = alT1m

            cb_toggle = [0]
            fstate = {}

            def copyback(dst_ap, src_ap):
                if cb_toggle[0] % 4 != 3:
                    nc.vector.tensor_copy(dst_ap, src_ap)
                else:
                    nc.scalar.activation(dst_ap, src_ap, AF.Copy)
                cb_toggle[0] += 1

            def xwork(t):
                if mode == "chain":
                    alTs.pop(t, None)
                    alTs.pop((t, '1m'), None)
                    return
                m, w = layers[t]
                alT = alTs.pop(t)
                if m < 128 and w >= 512:
                    # fused: accumulate per-tile [128,128] products, apply once
                    first = (t == 0) or not (layers[t - 1][0] < 128 and layers[t - 1][1] >= 512)
                    last = (t == NL - 1) or not (layers[t + 1][0] < 128 and layers[t + 1][1] >= 512)
                    ti = tmap[m]
                    psl = p_sb[:, ti * 128:(ti + 1) * 128]
                    dsl = dm_sb[:, ti * 128:(ti + 1) * 128]
                    for f in range(8):
                        if first:
                            ff = wp.tile([128, 128], F32R, tag=f"F{f}", name=f"F{f}_{t}", bufs=2)
                            fstate[f] = ff
                            nc.vector.tensor_scalar(ff[:], dsl, alT[:, f:f + 1], None, ALU.mult)
                            nc.gpsimd.tensor_tensor(ff[:], ff[:], psl, ALU.add)
                        else:
                            wt = wp.tile([128, 128], F32R, tag="wt")
                            nc.vector.tensor_scalar(wt[:], dsl, alT[:, f:f + 1], None, ALU.mult)
                            nc.gpsimd.tensor_tensor(wt[:], wt[:], psl, ALU.add)
                            ps = psmm.tile([128, 128], F32, tag="mm")
                            nc.tensor.matmul(ps[:], wt[:], fstate[f][:], start=True, stop=True)
                            copyback(fstate[f][:], ps[:])
                        if last:
                            pst = psmm.tile([128, 128], F32R, tag="mm")
                            nc.tensor.transpose(pst[:], fstate[f][:], idr_sb[:])
                            ft = wp.tile([128, 128], F32R, tag="wt")
                            nc.vector.tensor_copy(ft[:], pst[:])
                            off = w * ((128 * f) // w)
                            for c0 in range(off, off + w, 512):
                                cw = min(512, off + w - c0)
                                ps = psmm.tile([128, cw], F32, tag="mm")
                                nc.tensor.matmul(ps[:], ft[:], xt[f][:, c0:c0 + cw], start=True, stop=True)
                                copyback(xt[f][:, c0:c0 + cw], ps[:])
                elif m < 128:
                    ti = tmap[m]
                    psl = p_sb[:, ti * 128:(ti + 1) * 128]
                    dsl = dm_sb[:, ti * 128:(ti + 1) * 128]
                    for f in range(8):
                        wt = wp.tile([128, 128], F32R, tag="wt")
                        nc.vector.tensor_scalar(wt[:], dsl, alT[:, f:f + 1], None, ALU.mult)
                        nc.gpsimd.tensor_tensor(wt[:], wt[:], psl, ALU.add)
                        off = w * ((128 * f) // w)
                        for c0 in range(off, off + w, 512):
                            cw = min(512, off + w - c0)
                            ps = psmm.tile([128, cw], F32, tag="mm")
                            nc.tensor.matmul(ps[:], wt[:], xt[f][:, c0:c0 + cw], start=True, stop=True)
                            copyback(xt[f][:, c0:c0 + cw], ps[:])
                else:
                    k = m // 128
                    alT1m = alTs.pop((t, '1m'))
                    done = set()
                    for f in range(8):
                        f2 = f ^ k
                        if f in done:
                            continue
                        done.update((f, f2))
                        v1a = wp.tile([128, 128], F32R, tag="wt")
                        v2a = wp.tile([128, 128], F32R, tag="wt")
                        v1b = wp.tile([128, 128], F32R, tag="wt")
                        v2b = wp.tile([128, 128], F32R, tag="wt")
                        nc.scalar.activation(v1a[:], idr_sb[:], AF.Copy, scale=alT[:, f:f + 1])
                        nc.vector.tensor_scalar(v2a[:], idr_sb[:], alT1m[:, f:f + 1], None, ALU.mult)
                        nc.scalar.activation(v1b[:], idr_sb[:], AF.Copy, scale=alT[:, f2:f2 + 1])
                        nc.vector.tensor_scalar(v2b[:], idr_sb[:], alT1m[:, f2:f2 + 1], None, ALU.mult)
                        off = w * ((128 * f) // w)
                        for c0 in range(off, off + w, 512):
                            cw = min(512, off + w - c0)
                            psa = psmm.tile([128, cw], F32, tag="mm")
                            nc.tensor.matmul(psa[:], v1a[:], xt[f][:, c0:c0 + cw], start=True, stop=False)
                            nc.tensor.matmul(psa[:], v2a[:], xt[f2][:, c0:c0 + cw], start=False, stop=True)
                            psb = psmm.tile([128, cw], F32, tag="mm")
                            nc.tensor.matmul(psb[:], v1b[:], xt[f2][:, c0:c0 + cw], start=True, stop=False)
                            nc.tensor.matmul(psb[:], v2b[:], xt[f][:, c0:c0 + cw], start=False, stop=True)
                            copyback(xt[f][:, c0:c0 + cw], psa[:])
                            copyback(xt[f2][:, c0:c0 + cw], psb[:])

            chain(0)
            chain(1)
            chain(2)
            for t in range(NL):
                if t + 3 < NL:
                    chain(t + 3)
                xwork(t)

            nc.sync.dma_start(out=xs_d[:], in_=x8[:])
            for f in range(8):
                nc.sync.dma_start(
                    out=xt_d[f * 128:(f + 1) * 128, :],
                    in_=xt[f][:].bitcast(F32),
                )

    nc.compile()
    return nc


def kernel(vectors, idx_a, idx_b):
    from concourse.bass_utils import run_bass_kernel_spmd

    vectors = np.asarray(vectors, dtype=np.float32)
    ia = np.asarray(idx_a).astype(np.int64)
    ib = np.asarray(idx_b).astype(np.int64)

    layers = _derive_layers(ia, ib)
    key = (ia.tobytes(), ib.tobytes())
    if key not in _CACHE:
        _CACHE[key] = _build_program(layers)
    nc = _CACHE[key]

    sgn_r, pm_r = _build_masks(ia, ib, layers)
    small_ms = sorted({m for (m, _) in layers if m < 128})
    NT = len(small_ms)
    pmask = np.zeros((128, NT, 128), np.float32)
    dmask = np.zeros((128, NT, 128), np.float32)
    eye = np.eye(128, dtype=np.float32)
    for i, m in enumerate(small_ms):
        p = np.arange(128)
        pmat = np.zeros((128, 128), np.float32)
        pmat[p, p ^ m] = 1.0
        pmask[:, i, :] = pmat
        dmask[:, i, :] = eye - pmat
    pmask = pmask.reshape(128, NT * 128)
    dmask = dmask.reshape(128, NT * 128)

    # pair-difference matrices (transposed for lhsT): dfull = MD @ xT
    mdm = np.zeros((NT, 128, 128), np.float32)
    for i, m in enumerate(small_ms):
        for p in range(128):
            mdm[i, p, p | m] += 1.0
            mdm[i, p, p & ~m] -= 1.0
    mdm_t = mdm.transpose(0, 2, 1).transpose(1, 0, 2).reshape(128, NT * 128).copy()

    base = {
        "sgn": sgn_r,
        "pm": pm_r,
        "pmask": pmask,
        "dmask": dmask,
        "id128": eye,
        "mdm": mdm_t,
    }
    in_maps = []
    for c in range(NCORES):
        m = dict(base)
        m["vec"] = vectors[c].reshape(8, 128).T.copy()
        in_maps.append(m)

    LAST_INMAP[0] = in_maps[0]
    res = run_bass_kernel_spmd(nc, in_maps, list(range(NCORES)))
    LAST_EXEC_NS[0] = res.exec_time_ns
    LAST_RESULTS[0] = res

    x_out = np.empty((NCORES, N), np.float32)
    X_out = np.empty((NCORES, N, N), np.float32)
    for c in range(NCORES):
        x_out[c] = res.results[c]["xs"].T.reshape(N)
        X_out[c] = res.results[c]["xt"].T
    return x_out, X_out


# revision 40
# speedup vs baseline: 1.2120x; 1.0575x over previous
import numpy as np

# DiffSortNet bitonic differentiable sort, B=8, N=1024, 55 layers.
# Strategy: data-parallel (one batch element per core). Per core, the
# permutation matrix is kept TRANSPOSED in SBUF as 8 tiles XT_f [128,1024]
# (partition = column index of X). Each layer is a per-column blend
#   new_col_c = al_c * col_c + (1-al_c) * col_{c^m}
# with pair-symmetric alpha, which in the transposed layout is a [128,128]
# matmul per tile with the symmetric blend matrix Wt = al*D + P  (D = I-P,
# P the XOR-m permutation), or for m>=128 a pair of diagonal-matrix matmuls
# across tile pairs. Alphas come from a tiny [8,128] x-chain computed on
# DVE/ACT and transposed to per-partition scale vectors via the PE.

N = 1024
NL = 55
NCORES = 8

_CACHE = {}
LAST_EXEC_NS = [None]
LAST_RESULTS = [None]
LAST_INMAP = [None]


def _derive_layers(ia, ib):
    ms = (ia ^ ib)
    layers = []
    prev_m = None
    cur_block_top = None
    for t in range(ia.shape[0]):
        mv = np.unique(ms[t])
        assert mv.size == 1, "non-uniform stride in layer"
        m = int(mv[0])
        if prev_m is None or m > prev_m:
            cur_block_top = m
        s_out = 2 * cur_block_top
        w = min(max(s_out, 128), N)
        layers.append((m, w))
        prev_m = m
    return layers


def _build_masks(ia, ib, layers):
    # sgn10[t, c] = +10 if ib of c's pair sits at (c|m) else -10
    # pm[t, c]    = +1 if c is the low member of its pair else -1
    sgn = np.empty((NL, N), np.float32)
    pm = np.empty((NL, N), np.float32)
    for t, (m, _) in enumerate(layers):
        a, b = ia[t], ib[t]
        hi = np.maximum(a, b)
        val = np.where(b == hi, 10.0, -10.0).astype(np.float32)
        s = np.empty(N, np.float32)
        s[a] = val
        s[b] = val
        sgn[t] = s
        c = np.arange(N)
        pm[t] = np.where((c & m) == 0, 1.0, -1.0)
    # reshape to [128, NL*8]: row p, col (t, f), c = f*128 + p
    sgn_r = sgn.reshape(NL, 8, 128).transpose(2, 0, 1).reshape(128, NL * 8).copy()
    pm_r = pm.reshape(NL, 8, 128).transpose(2, 0, 1).reshape(128, NL * 8).copy()
    return sgn_r, pm_r


def _build_program(layers):
    import os
    import concourse.bacc as bacc
    import concourse.mybir as mybir
    from concourse.tile import TileContext

    mode = os.environ.get("DSORT_MODE", "full")

    F32 = mybir.dt.float32
    F32R = mybir.dt.float32r
    AF = mybir.ActivationFunctionType
    ALU = mybir.AluOpType

    small_ms = sorted({m for (m, _) in layers if m < 128})
    tmap = {m: i for i, m in enumerate(small_ms)}
    NT = len(small_ms)

    nc = bacc.Bacc()
    vec_d = nc.dram_tensor("vec", [128, 8], F32, kind="ExternalInput")
    sgn_d = nc.dram_tensor("sgn", [128, NL * 8], F32, kind="ExternalInput")
    pm_d = nc.dram_tensor("pm", [128, NL * 8], F32, kind="ExternalInput")
    p_d = nc.dram_tensor("pmask", [128, NT * 128], F32, kind="ExternalInput")
    d_d = nc.dram_tensor("dmask", [128, NT * 128], F32, kind="ExternalInput")
    id128_d = nc.dram_tensor("id128", [128, 128], F32, kind="ExternalInput")
    md_d = nc.dram_tensor("mdm", [128, NT * 128], F32, kind="ExternalInput")
    xs_d = nc.dram_tensor("xs", [128, 8], F32, kind="ExternalOutput")
    xt_d = nc.dram_tensor("xt", [N, N], F32, kind="ExternalOutput")

    with TileContext(nc) as tc:
        with tc.tile_pool(name="cst", bufs=1) as cst, \
             tc.tile_pool(name="xp", bufs=1) as xp, \
             tc.tile_pool(name="ch", bufs=4) as ch, \
             tc.tile_pool(name="wp", bufs=8) as wp, \
             tc.tile_pool(name="psmm", bufs=4, space="PSUM") as psmm, \
             tc.tile_pool(name="pstr", bufs=2, space="PSUM") as pstr:

            sgn_sb = cst.tile([128, NL * 8], F32, tag="sgn")
            pm_sb = cst.tile([128, NL * 8], F32, tag="pmm")
            p_sb = cst.tile([128, NT * 128], F32R, tag="pall")
            dm_sb = cst.tile([128, NT * 128], F32R, tag="dall")
            idr_sb = cst.tile([128, 128], F32R, tag="idr")
            md_sb = cst.tile([128, NT * 128], F32, tag="mdm")
            nc.sync.dma_start(out=md_sb[:], in_=md_d[:])
            epsb = cst.tile([128, 1], F32, tag="epsb")
            nc.gpsimd.memset(epsb[:], 1e-38)
            nc.sync.dma_start(out=sgn_sb[:], in_=sgn_d[:])
            nc.sync.dma_start(out=pm_sb[:], in_=pm_d[:])
            nc.sync.dma_start(out=p_sb[:], in_=p_d[:].bitcast(F32R))
            nc.sync.dma_start(out=dm_sb[:], in_=d_d[:].bitcast(F32R))
            nc.sync.dma_start(out=idr_sb[:], in_=id128_d[:].bitcast(F32R))

            xt = [xp.tile([128, N], F32R, tag=f"X{f}", name=f"X{f}") for f in range(8)]
            zsc = cst.tile([128, N], F32, tag="zsc")
            nc.gpsimd.memset(zsc[:], 0.0)
            for f in range(8):
                if f % 2 == 0:
                    nc.gpsimd.tensor_copy(xt[f][:], zsc[:])
                elif f % 4 == 1:
                    nc.vector.tensor_copy(xt[f][:], zsc[:])
                else:
                    nc.scalar.activation(xt[f][:], zsc[:], AF.Copy)
                nc.sync.dma_start(
                    out=xt[f][:, f * 128:(f + 1) * 128],
                    in_=id128_d[:].bitcast(F32R),
                )

            x8 = ch.tile([128, 8], F32, tag="x8", bufs=4)
            nc.gpsimd.dma_start(out=x8[:], in_=vec_d[:])

            alTs = {}

            alTc = cst.tile([128, 8], F32, tag="alTc")
            nc.gpsimd.memset(alTc[:], 0.5)

            def chain(t):
                nonlocal x8
                m, w = layers[t]
                if mode == "xwork":
                    alTs[t] = alTc
                    if m >= 128:
                        alTs[(t, '1m')] = alTc
                    return
                if m < 128:
                    # pair difference across partitions via const matrix matmul;
                    # chain consumers read it straight from PSUM (no copy hop)
                    ti = tmap[m]
                    dps = pstr.tile([128, 8], F32, tag="dps", bufs=2)
                    nc.tensor.matmul(dps[:], md_sb[:, ti * 128:(ti + 1) * 128], x8[:], start=True, stop=True)
                    dfull = dps
                else:
                    dfull = ch.tile([128, 8], F32, tag="dfull")
                    k = m // 128
                    xv = x8[:].rearrange("p (g two k) -> p g two k", two=2, k=k)
                    dv = dfull[:].rearrange("p (g two k) -> p g two k", two=2, k=k)
                    hi = xv[:, :, 1:2, :].broadcast_to((128, 8 // (2 * k), 2, k))
                    lo = xv[:, :, 0:1, :].broadcast_to((128, 8 // (2 * k), 2, k))
                    nc.vector.tensor_tensor(dv[:, :, :, :], hi, lo, ALU.subtract)
                # critical recurrence: dfull -> Abs -> Ln -> Exp(.75) -> z3 -> sig- -> q -> x8n
                a1 = ch.tile([128, 8], F32, tag="a1")
                nc.scalar.activation(a1[:], dfull[:], AF.Abs)
                lg = ch.tile([128, 8], F32, tag="lg")
                nc.scalar.activation(lg[:], a1[:], AF.Ln, bias=epsb[:])
                w34 = ch.tile([128, 8], F32, tag="w34")
                nc.scalar.activation(w34[:], lg[:], AF.Exp, scale=0.75)
                # parallel branches (off critical path)
                s0m0 = ch.tile([128, 8], F32, tag="s0m0")
                nc.vector.tensor_tensor(s0m0[:], dfull[:], sgn_sb[:, t * 8:(t + 1) * 8], ALU.mult)
                s0m = ch.tile([128, 8], F32, tag="s0m")
                nc.scalar.activation(s0m[:], s0m0[:], AF.Sign)
                t1 = ch.tile([128, 8], F32, tag="t1")
                nc.vector.tensor_tensor(t1[:], dfull[:], pm_sb[:, t * 8:(t + 1) * 8], ALU.mult)
                # join
                z3 = ch.tile([128, 8], F32, tag="z3")
                nc.vector.tensor_tensor(z3[:], w34[:], s0m[:], ALU.mult)
                sigm = ch.tile([128, 8], F32, tag="sigm")
                nc.scalar.activation(sigm[:], z3[:], AF.Sigmoid, scale=-10.0)
                q = ch.tile([128, 8], F32, tag="q")
                nc.vector.tensor_tensor(q[:], t1[:], sigm[:], ALU.mult)
                x8n = ch.tile([128, 8], F32, tag="x8", bufs=4)
                nc.vector.tensor_tensor(x8n[:], x8[:], q[:], ALU.add)
                x8 = x8n
                # alpha for the X update (off recurrence) -- already [128, 8]
                alT = ch.tile([128, 8], F32, tag="alT", bufs=6)
                nc.scalar.activation(alT[:], z3[:], AF.Sigmoid, scale=10.0)
                alTs[t] = alT
                if m >= 128:
                    alT1m = ch.tile([128, 8], F32, tag="alT1m")
                    nc.scalar.activation(alT1m[:], sigm[:], AF.Copy)
                    alTs[(t, '1m')] = alT1m

            cb_toggle = [0]
            fstate = {}

            def copyback(dst_ap, src_ap):
                if cb_toggle[0] % 4 != 3:
                    nc.vector.tensor_copy(dst_ap, src_ap)
                else:
                    nc.scalar.activation(dst_ap, src_ap, AF.Copy)
                cb_toggle[0] += 1

            def xwork(t):
                if mode == "chain":
                    alTs.pop(t, None)
                    alTs.pop((t, '1m'), None)
                    return
                m, w = layers[t]
                alT = alTs.pop(t)
                if m < 128 and w >= 512:
                    # fused: accumulate per-tile [128,128] products, apply once
                    first = (t == 0) or not (layers[t - 1][0] < 128 and layers[t - 1][1] >= 512)
                    last = (t == NL - 1) or not (layers[t + 1][0] < 128 and layers[t + 1][1] >= 512)
                    ti = tmap[m]
                    psl = p_sb[:, ti * 128:(ti + 1) * 128]
                    dsl = dm_sb[:, ti * 128:(ti + 1) * 128]
                    for f in range(8):
                        if first:
                            ff = wp.tile([128, 128], F32R, tag=f"F{f}", name=f"F{f}_{t}", bufs=2)
                            fstate[f] = ff
                            nc.vector.tensor_scalar(ff[:], dsl, alT[:, f:f + 1], None, ALU.mult)
                            nc.gpsimd.tensor_tensor(ff[:], ff[:], psl, ALU.add)
                        else:
                            wt = wp.tile([128, 128], F32R, tag="wt")
                            nc.vector.tensor_scalar(wt[:], dsl, alT[:, f:f + 1], None, ALU.mult)
                            nc.gpsimd.tensor_tensor(wt[:], wt[:], psl, ALU.add)
                            ps = psmm.tile([128, 128], F32, tag="mm")
                            nc.tensor.matmul(ps[:], wt[:], fstate[f][:], start=True, stop=True)
                            copyback(fstate[f][:], ps[:])
                        if last:
                            pst = psmm.tile([128, 128], F32R, tag="mm")
                            nc.tensor.transpose(pst[:], fstate[f][:], idr_sb[:])
                            ft = wp.tile([128, 128], F32R, tag="wt")
                            nc.vector.tensor_copy(ft[:], pst[:])
                            off = w * ((128 * f) // w)
                            for c0 in range(off, off + w, 512):
                                cw = min(512, off + w - c0)
                                ps = psmm.tile([128, cw], F32, tag="mm")
                                nc.tensor.matmul(ps[:], ft[:], xt[f][:, c0:c0 + cw], start=True, stop=True)
                                copyback(xt[f][:, c0:c0 + cw], ps[:])
                elif m < 128:
                    ti = tmap[m]
                    psl = p_sb[:, ti * 128:(ti + 1) * 128]
                    dsl = dm_sb[:, ti * 128:(ti + 1) * 128]
                    for f in range(8):
                        wt = wp.tile([128, 128], F32R, tag="wt")
                        nc.vector.tensor_scalar(wt[:], dsl, alT[:, f:f + 1], None, ALU.mult)
                        nc.gpsimd.tensor_tensor(wt[:], wt[:], psl, ALU.add)
                        off = w * ((128 * f) // w)
                        for c0 in range(off, off + w, 512):
                            cw = min(512, off + w - c0)
                            ps = psmm.tile([128, cw], F32, tag="mm")
                            nc.tensor.matmul(ps[:], wt[:], xt[f][:, c0:c0 + cw], start=True, stop=True)
                            copyback(xt[f][:, c0:c0 + cw], ps[:])
                else:
                    k = m // 128
                    alT1m = alTs.pop((t, '1m'))
                    done = set()
                    for f in range(8):
                        f2 = f ^ k
                        if f in done:
                            continue
                        done.update((f, f2))
                        v1a = wp.tile([128, 128], F32R, tag="wt")
                        v2a = wp.tile([128, 128], F32R, tag="wt")
                        v1b = wp.tile([128, 128], F32R, tag="wt")
                        v2b = wp.tile([128, 128], F32R, tag="wt")
                        nc.scalar.activation(v1a[:], idr_sb[:], AF.Copy, scale=alT[:, f:f + 1])
                        nc.vector.tensor_scalar(v2a[:], idr_sb[:], alT1m[:, f:f + 1], None, ALU.mult)
                        nc.scalar.activation(v1b[:], idr_sb[:], AF.Copy, scale=alT[:, f2:f2 + 1])
                        nc.vector.tensor_scalar(v2b[:], idr_sb[:], alT1m[:, f2:f2 + 1], None, ALU.mult)
                        off = w * ((128 * f) // w)
                        for c0 in range(off, off + w, 512):
                            cw = min(512, off + w - c0)
                            psa = psmm.tile([128, cw], F32, tag="mm")
                            nc.tensor.matmul(psa[:], v1a[:], xt[f][:, c0:c0 + cw], start=True, stop=False)
                            nc.tensor.matmul(psa[:], v2a[:], xt[f2][:, c0:c0 + cw], start=False, stop=True)
                            psb = psmm.tile([128, cw], F32, tag="mm")
                            nc.tensor.matmul(psb[:], v1b[:], xt[f2][:, c0:c0 + cw], start=True, stop=False)
                            nc.tensor.matmul(psb[:], v2b[:], xt[f][:, c0:c0 + cw], start=False, stop=True)
                            copyback(xt[f][:, c0:c0 + cw], psa[:])
                            copyback(xt[f2][:, c0:c0 + cw], psb[:])

            chain(0)
            chain(1)
            chain(2)
            for t in range(NL):
                if t + 3 < NL:
                    chain(t + 3)
                xwork(t)

            nc.sync.dma_start(out=xs_d[:], in_=x8[:])
            for f in range(8):
                nc.sync.dma_start(
                    out=xt_d[f * 128:(f + 1) * 128, :],
                    in_=xt[f][:].bitcast(F32),
                )

    nc.compile()
    return nc


def kernel(vectors, idx_a, idx_b):
    from concourse.bass_utils import run_bass_kernel_spmd

    vectors = np.asarray(vectors, dtype=np.float32)
    ia = np.asarray(idx_a).astype(np.int64)
    ib = np.asarray(idx_b).astype(np.int64)

    layers = _derive_layers(ia, ib)
    key = (ia.tobytes(), ib.tobytes())
    if key not in _CACHE:
        _CACHE[key] = _build_program(layers)
    nc = _CACHE[key]

    sgn_r, pm_r = _build_masks(ia, ib, layers)
    small_ms = sorted({m for (m, _) in layers if m < 128})
    NT = len(small_ms)
    pmask = np.zeros((128, NT, 128), np.float32)
    dmask = np.zeros((128, NT, 128), np.float32)
    eye = np.eye(128, dtype=np.float32)
    for i, m in enumerate(small_ms):
        p = np.arange(128)
        pmat = np.zeros((128, 128), np.float32)
        pmat[p, p ^ m] = 1.0
        pmask[:, i, :] = pmat
        dmask[:, i, :] = eye - pmat
    pmask = pmask.reshape(128, NT * 128)
    dmask = dmask.reshape(128, NT * 128)

    # pair-difference matrices (transposed for lhsT): dfull = MD @ xT
    mdm = np.zeros((NT, 128, 128), np.float32)
    for i, m in enumerate(small_ms):
        for p in range(128):
            mdm[i, p, p | m] += 1.0
            mdm[i, p, p & ~m] -= 1.0
    mdm_t = mdm.transpose(0, 2, 1).transpose(1, 0, 2).reshape(128, NT * 128).copy()

    base = {
        "sgn": sgn_r,
        "pm": pm_r,
        "pmask": pmask,
        "dmask": dmask,
        "id128": eye,
        "mdm": mdm_t,
    }
    in_maps = []
    for c in range(NCORES):
        m = dict(base)
        m["vec"] = vectors[c].reshape(8, 128).T.copy()
        in_maps.append(m)

    LAST_INMAP[0] = in_maps[0]
    res = run_bass_kernel_spmd(nc, in_maps, list(range(NCORES)))
    LAST_EXEC_NS[0] = res.exec_time_ns
    LAST_RESULTS[0] = res

    x_out = np.empty((NCORES, N), np.float32)
    X_out = np.empty((NCORES, N, N), np.float32)
    for c in range(NCORES):
        x_out[c] = res.results[c]["xs"].T.reshape(N)
        X_out[c] = res.results[c]["xt"].T
    return x_out, X_out
